# revision 33
# baseline (speedup 1.0000x reference)
"""GPT2-style fused attention (DecisionTransformer) on 8 Trainium2 NeuronCores.

Sharding: tensor-parallel over the 16 heads (2 heads per core, both batch
elements on every core).  v4 changes vs v3:
  - causal diagonal mask is a multiplicative 0/1 mask applied after exp on
    the (otherwise idle) GpSimd engine (GpSimd cannot touch PSUM, so the
    additive-PSUM-seed idea is out); pt lives in SBUF so this is legal.
  - ones-BLOCK denominator: V_aug columns [64,128) are all ones, so the
    A@V matmul itself materializes the softmax denominator broadcast
    across PSUM partitions 64-127 at zero extra cycles.  This removes the
    per-chunk DVE den-copy and the PE rank-1 broadcast matmul.
  - normalize (DVE) and projection (PE) for query-chunk qc are deferred
    and emitted inside qc+1's key-block loop, so the projection matmuls
    fill PE stalls while exp catches up, instead of stalling the PE queue
    at the chunk boundary.
v3: row-tiled concurrent per-head score matmuls (64-row contraction at
    tile_position (0,0)/(64,0)), Q in natural stacked layout, A@V software-
    pipelined one key block behind exp.
v2: host-side X^T in bf16, bf16 weights/matmuls everywhere, merged
    two-head exp, bf16 partial outputs summed on host in fp32.
"""

import sys

for _p in ("/opt/trn_rl_repo",):
    if _p not in sys.path:
        sys.path.insert(0, _p)

import numpy as np

import concourse.bass as bass
import concourse.mybir as mybir
import concourse.tile as tile
from concourse import bacc
from concourse.bass_utils import run_bass_kernel_spmd
from concourse.masks import make_identity

P = 128
B, S, D, H, HD = 2, 2048, 1024, 16, 64
T = B * S              # 4096 tokens
FQKV = 3 * P           # 384 per-core qkv features (q128 | k128 | v128)
KO = D // P            # 8 contraction chunks
TCH = 512              # token chunk for qkv phase
QC = 512               # query chunk in attention
NQC = S // QC          # 4
NKB = S // P           # 16 key blocks per sequence
SCALE = 1.0 / float(HD) ** 0.5
N_CORES = 8
HPC = H // N_CORES     # 2 heads per core

f32 = mybir.dt.float32
f32r = mybir.dt.float32r
bf16 = mybir.dt.bfloat16
MM_DT = bf16
# Row-tiled per-head score matmuls (64-row contraction, concurrent PE array
# halves).  Fallback (False): zero-padded Q, full-128 contraction.
ROW_TILED = True


def _emit_body(nc, tc, pools, consts, it, phases='full'):
    (xt_pool, qkvt_pool, vaug_pool, pt_pool, atn_pool, out_pool,
     small_pool, ps_mm, ps_s, ps_o) = pools
    (wqkv_sb, wp_sb, bqkv_sb, ident2, mask128, x_d, out_d, xt_pre) = consts

    ktb = [qkvt_pool.tile([P, S], MM_DT, tag=f"kt{b}", name=f"kt{b}")
           for b in range(B)]
    vtb = [qkvt_pool.tile([P, S], f32r, tag=f"vt{b}", name=f"vt{b}")
           for b in range(B)]
    if ROW_TILED:
        # Q^T natural stacked layout [h0 64 | h1 64, S] -- same as K^T; the
        # row-tiled score matmul contracts each head's 64 rows separately.
        qtb = [qkvt_pool.tile([P, S], MM_DT, tag=f"qt{b}", name=f"qt{b}")
               for b in range(B)]
    else:
        # Q^T per (batch, local head), zero-padded to 128 contraction rows
        qpad = [
            [qkvt_pool.tile([P, S], MM_DT, tag=f"qp{b}{h}", name=f"qp{b}{h}")
             for h in range(HPC)]
            for b in range(B)
        ]
        if it == 0:
            for b in range(B):
                nc.gpsimd.memset(qpad[b][0][HD:, :], 0.0)
                nc.gpsimd.memset(qpad[b][1][:HD, :], 0.0)
    vaug = [
        vaug_pool.tile([P, NKB, P], MM_DT, tag=f"vaug{p}", name=f"vaug{p}")
        for p in range(B * HPC)
    ]
    atn = [
        [
            atn_pool.tile([P, QC], MM_DT, tag=f"atn{b}_{q}", name=f"atn{b}_{q}")
            for q in range(NQC)
        ]
        for b in range(B)
    ]

    # V_aug layout: columns [0,64) are all ones and columns [64,128) hold V
    # in natural layout -- the A@V matmul then materializes the softmax
    # denominator broadcast across PSUM partitions 0-63 at zero extra
    # cycles.  Ones-first, because the custom-DVE reciprocal misreads PSUM
    # at base partition 64.
    def emit_vaug_pair(b, hl, kb, psum_pool):
        p = b * HPC + hl
        vt = vtb[b][hl * HD : (hl + 1) * HD, :]
        if psum_pool == 's':
            ps = ps_s.tile([P, HPC, QC], f32, tag="s", name="pss")[:, 0, :]
        else:
            ps = ps_mm.tile([P, TCH], f32, tag="mm", name="psmm")
        for u in range(2):
            nc.tensor.transpose(
                ps[:, u * HD : (u + 1) * HD].bitcast(f32r),
                vt[:, (kb + u) * P : (kb + u + 1) * P],
                ident2[hl * HD : (hl + 1) * HD, :],
            )
        nc.vector.tensor_copy(
            vaug[p][:, kb : kb + 2, HD:],
            ps[:, : 2 * HD].rearrange("p (u h) -> p u h", u=2),
        )

    # ---- QKV projection with the V_aug transpose pairs folded into each
    # chunk (transpose-mode doesn't count as PE-busy for the HAM clock
    # gate; keeping them adjacent to the matmul stream avoids a
    # transpose-only stretch that would re-throttle the PE) ----
    # hoist all chunk DMAs up front (xt pool holds all 8): a chunk DMA
    # issued lazily mid-attention can stall the in-order PE queue behind
    # its first matmul
    xts = []
    for b in range(B):
        for i in range(S // TCH):
            gi = b * (S // TCH) + i
            if it == 0 and gi == 0:
                xts.append(xt_pre)
            else:
                xt = xt_pool.tile([P, KO, TCH], MM_DT, tag=f"xt{gi}",
                                  name=f"xt{gi}")
                nc.sync.dma_start(
                    xt[:],
                    x_d.rearrange("(ko p) t -> p ko t", p=P)[
                        :, :, gi * TCH : (gi + 1) * TCH
                    ],
                )
                xts.append(xt)

    def gen_qkv():
        """Emit all 8 QKV chunks; yields ('step',) between work packets and
        ('done', b, i) after chunk (b, i) -- the attention loops drain to
        the marker they need (flash-style gating: query chunk qc of batch b
        only needs K/V chunks <= qc of that batch)."""
        for b in range(B):
            if it == 0:
                for hl in range(HPC):
                    nc.gpsimd.memset(vaug[b * HPC + hl][:, :, :HD], 1.0)
            for i in range(S // TCH):
                gi = b * (S // TCH) + i
                # chunk (0,0) runs up front with ps_s free for V_aug;
                # every other chunk interleaves into attention (ps_s hot)
                vaug_pool = 's' if gi == 0 else 'mm'
                xt = xts[gi]
                yield ('step',)
                for fc in range(3):
                    ps = ps_mm.tile([P, TCH], f32, tag="mm", name="psmm")
                    for ko in range(KO):
                        nc.tensor.matmul(
                            ps[:],
                            wqkv_sb[:, ko, fc * P : (fc + 1) * P],
                            xt[:, ko, :],
                            start=(ko == 0),
                            stop=(ko == KO - 1),
                        )
                        if ko % 2 == 1:
                            yield ('step',)
                    # evict + per-partition bias add on DVE
                    cs = slice(i * TCH, (i + 1) * TCH)
                    if fc == 0 and not ROW_TILED:
                        nc.vector.tensor_scalar(
                            qpad[b][0][:HD, cs], ps[:HD],
                            bqkv_sb[:HD, fc : fc + 1], None,
                            mybir.AluOpType.add,
                        )
                        nc.vector.tensor_scalar(
                            qpad[b][1][HD:, cs], ps[HD:],
                            bqkv_sb[HD:, fc : fc + 1], None,
                            mybir.AluOpType.add,
                        )
                    else:
                        dst = (qtb[b] if ROW_TILED else None,
                               ktb[b], vtb[b])[fc]
                        nc.vector.tensor_scalar(
                            dst[:, cs], ps[:],
                            bqkv_sb[:, fc : fc + 1], None,
                            mybir.AluOpType.add,
                        )
                    yield ('step',)
                # V_aug pairs for the key blocks this chunk just produced
                for hl in range(HPC):
                    for kb in (4 * i, 4 * i + 2):
                        emit_vaug_pair(b, hl, kb, vaug_pool)
                        yield ('step',)
                yield ('done', b, i)

    qkv = gen_qkv()
    qkv_done = set()

    def qkv_step(n=1):
        for _ in range(n):
            m = next(qkv, None)
            if m is not None and m[0] == 'done':
                qkv_done.add((m[1], m[2]))

    def qkv_drain_until(b, i):
        while (b, i) not in qkv_done:
            m = next(qkv)
            if m[0] == 'done':
                qkv_done.add((m[1], m[2]))

    # chunk (0,0) up front: batch 0's first query chunk needs it
    qkv_drain_until(0, 0)

    if phases == 'a':
        for _ in qkv:
            pass
        return

    # ---- phase 3+4: attention + output projection ----
    def make_proj(b, qc, final=False):
        def emit_proj():
            # merged store: projection evictions land in a staging tile
            # written by one wide DMA.  The final chunk instead uses two
            # half-stores with evictions alternating DVE/Act so the
            # end-of-kernel drain pipelines instead of serializing.
            halves = 2 if final else 1
            span = (QC // P) // halves
            for h in range(halves):
                ot = out_pool.tile([P, QC // P, D], MM_DT, tag="ot",
                                   name="ot")
                for qi in range(span):
                    qb = h * span + qi
                    for nck in range(2):
                        pp = ps_mm.tile([P, TCH], f32, tag="mm", name="psmm")
                        nc.tensor.matmul(
                            pp[:, :512],
                            atn[b][qc][:, qb * P : (qb + 1) * P],
                            wp_sb[:, nck * 512 : (nck + 1) * 512],
                            start=True,
                            stop=True,
                        )
                        dst = ot[:, qi, nck * 512 : (nck + 1) * 512]
                        if final and (qb + nck) % 2:
                            nc.scalar.copy(dst, pp[:, :512])
                        else:
                            nc.vector.tensor_copy(dst, pp[:, :512])
                row = b * S + qc * QC + h * span * P
                nc.sync.dma_start(
                    out_d[row : row + span * P, :].rearrange(
                        "(qb p) d -> p qb d", p=P
                    ),
                    ot[:, :span, :],
                )
        return emit_proj

    prev_proj = None
    for b in range(B):
        # batch 1 runs largest-chunk-first: its qc3 gate drains the rest of
        # the QKV generator as one PE-dense warm burst at the phase
        # boundary, and the kernel ends on the smallest chunk (qc0, 4 key
        # blocks) whose exp-pacing deficit its projection filler covers --
        # instead of starving through qc3's 16 blocks and re-throttling
        # into the drain
        for qc in (range(NQC) if b == 0 else range(NQC - 1, -1, -1)):
            # flash-style gate: this query chunk attends keys < (qc+1)*512,
            # so K/V chunks <= qc of this batch must be complete
            qkv_drain_until(b, qc)
            po = [
                ps_o.tile([P, QC], f32, tag=f"po{hl}", name=f"pso{hl}")
                for hl in range(HPC)
            ]
            nkb = (qc + 1) * (QC // P)

            def emit_av(kb, pt2, lo):
                for hl in range(HPC):
                    nc.tensor.matmul(
                        po[hl][:, lo:],
                        vaug[b * HPC + hl][:, kb, :],
                        pt2[:, hl, lo:],
                        start=(kb == 0),
                        stop=(kb == nkb - 1),
                    )

            pending = None      # (kb, pt2, lo): A@V trails exp by one block
            for kb in range(nkb):
                j = kb - qc * (QC // P)
                lo = j * P if j > 0 else 0
                ps2 = ps_s.tile([P, HPC, QC], f32, tag="s", name="pss")
                for hl in range(HPC):
                    if ROW_TILED:
                        # row-tiled: head hl contracts rows [64*hl, ...+64);
                        # the two heads run concurrently in PE array halves
                        nc.tensor.matmul(
                            ps2[:, hl, lo:],
                            ktb[b][hl * HD : (hl + 1) * HD,
                                   kb * P : (kb + 1) * P],
                            qtb[b][hl * HD : (hl + 1) * HD,
                                   qc * QC + lo : (qc + 1) * QC],
                            start=True,
                            stop=True,
                        )
                    else:
                        nc.tensor.matmul(
                            ps2[:, hl, lo:],
                            ktb[b][:, kb * P : (kb + 1) * P],
                            qpad[b][hl][:, qc * QC + lo : (qc + 1) * QC],
                            start=True,
                            stop=True,
                        )
                pt2 = pt_pool.tile([P, HPC, QC], MM_DT, tag="pt", name="pt")
                # one exp covers both heads (free size up to 1024)
                nc.scalar.activation(
                    pt2[:, :, lo:],
                    ps2[:, :, lo:],
                    mybir.ActivationFunctionType.Exp,
                    scale=SCALE,
                )
                if j >= 0:
                    # diagonal block: zero the strictly-upper triangle on
                    # the idle GpSimd engine (SBUF-only ops there)
                    for hl in range(HPC):
                        nc.gpsimd.tensor_tensor(
                            pt2[:, hl, j * P : (j + 1) * P],
                            pt2[:, hl, j * P : (j + 1) * P],
                            mask128[:],
                            mybir.AluOpType.mult,
                        )
                if pending is not None:
                    emit_av(*pending)
                # the previous chunk's projection slots in here: the PE
                # reaches it while exp for this chunk's early blocks runs
                if kb == 1 and prev_proj is not None:
                    prev_proj()
                    prev_proj = None
                # two QKV steps per key block: independent PE work that
                # fills exp stalls and keeps the HAM window busy
                qkv_step(2)
                pending = (kb, pt2, lo)
            emit_av(*pending)
            # normalize inline (pure DVE; the PE-side score stream of the
            # next chunk doesn't depend on it, and the early emission frees
            # the po banks before the next chunk's first A@V needs them)
            for hl in range(HPC):
                # po[0:64] holds the denominator broadcast across 64
                # partitions (ones-block trick); ~51 ULP reciprocal is
                # plenty for softmax denominators
                rbs = small_pool.tile([HD, QC], f32, tag="rbs", name="rbs")
                nc.vector.reciprocal_approx_fast(
                    out=rbs[:], in_=po[hl][:HD, :]
                )
                nc.vector.tensor_tensor(
                    atn[b][qc][hl * HD : (hl + 1) * HD, :],
                    po[hl][HD:, :],
                    rbs[:],
                    mybir.AluOpType.mult,
                )
            prev_proj = make_proj(b, qc, final=(b == B - 1 and qc == 0))
    for _ in qkv:
        pass
    prev_proj()


def _build_program(iters=1, phases='full'):
    nc = bacc.Bacc(None, target_bir_lowering=False)

    x_d = nc.dram_tensor("x", [D, T], bf16, kind="ExternalInput")
    wqkv_d = nc.dram_tensor("w_qkv", [D, FQKV], bf16, kind="ExternalInput")
    bqkv_d = nc.dram_tensor("b_qkv", [FQKV], f32, kind="ExternalInput")
    wp_d = nc.dram_tensor("w_proj", [P, D], bf16, kind="ExternalInput")
    out_d = nc.dram_tensor("out", [T, D], bf16, kind="ExternalOutput")

    with tile.TileContext(nc) as tc:
        with (
            tc.tile_pool(name="const", bufs=1) as const,
            tc.tile_pool(name="xt", bufs=1) as xt_pool,
            tc.tile_pool(name="qkvt", bufs=1) as qkvt_pool,
            tc.tile_pool(name="vaug", bufs=1) as vaug_pool,
            tc.tile_pool(name="pt", bufs=4) as pt_pool,
            tc.tile_pool(name="atn", bufs=1) as atn_pool,
            tc.tile_pool(name="outp", bufs=3) as out_pool,
            tc.tile_pool(name="small", bufs=3) as small_pool,
            tc.tile_pool(name="ps_mm", bufs=2, space="PSUM") as ps_mm,
            tc.tile_pool(name="ps_s", bufs=2, space="PSUM") as ps_s,
            tc.tile_pool(name="ps_o", bufs=1, space="PSUM") as ps_o,
        ):
            # ---- constants ----
            # prefetch the first token chunk before the weight DMAs so the
            # QKV pipeline starts immediately
            xt_pre = xt_pool.tile([P, KO, TCH], MM_DT, tag="xt", name="xt")
            nc.sync.dma_start(
                xt_pre[:],
                x_d.rearrange("(ko p) t -> p ko t", p=P)[:, :, 0:TCH],
            )
            wqkv_sb = const.tile([P, KO, FQKV], MM_DT)
            nc.sync.dma_start(
                wqkv_sb[:], wqkv_d.rearrange("(ko p) f -> p ko f", p=P)
            )
            wp_sb = const.tile([P, D], MM_DT)
            nc.sync.dma_start(wp_sb[:], wp_d[:])
            bqkv_sb = const.tile([P, 3], f32)
            nc.sync.dma_start(bqkv_sb[:], bqkv_d.rearrange("(c p) -> p c", p=P))
            # bf16 identity + a 512-wide scratch, built directly on GpSimd
            # (fast path: the warmups can start ~1.5us in)
            ident_bf = const.tile([P, P], MM_DT)
            nc.gpsimd.memset(ident_bf[:], 0.0)
            nc.gpsimd.affine_select(
                out=ident_bf[:],
                in_=ident_bf[:],
                compare_op=mybir.AluOpType.not_equal,
                fill=1.0,
                base=0,
                pattern=[[-1, P]],
                channel_multiplier=1,
            )
            wscr = const.tile([P, TCH], MM_DT)
            nc.gpsimd.memset(wscr[:], 0.5)
            # PE warmup: ~3.4us of sustained matmul activity releases the
            # HAM clock gate (1.2 -> 2.4 GHz); cover the initial DMA window
            # so the first real matmuls run at full clock.  Rotate across
            # three PSUM pools so write-after-write hazards on the pool
            # tiles don't put gaps in the activity window.
            for w in range(15):
                which = w % 3
                if which == 0:
                    wt = ps_s.tile([P, HPC, QC], f32, tag="s",
                                   name="pss")[:, 0, :]
                elif which == 1:
                    wt = ps_mm.tile([P, TCH], f32, tag="mm", name="psmm")
                else:
                    wt = ps_o.tile([P, QC], f32, tag="po0", name="pso0")
                nc.tensor.matmul(
                    wt[:], ident_bf[:], wscr[:], start=True, stop=True,
                )
            # ident2[r, c] = 1 iff r == c or r == c + 64 (c < 64): slices
            # [:64] / [64:] are 64x64 identities at partition base 0 / 64,
            # for transposing the per-head V^T chunks (lhsT and rhs of a
            # matmul must share the same base partition).
            ident2_f32 = const.tile([P, HD], f32)
            nc.gpsimd.memset(ident2_f32[:], 0.0)
            for base in (0, -HD):
                nc.gpsimd.affine_select(
                    out=ident2_f32[:],
                    in_=ident2_f32[:],
                    compare_op=mybir.AluOpType.not_equal,
                    fill=1.0,
                    base=base,
                    pattern=[[-1, HD]],
                    channel_multiplier=1,
                )
            ident2 = const.tile([P, HD], f32r)
            nc.vector.tensor_copy(ident2[:], ident2_f32[:])
            # mask128[k, q] = 1.0 if k <= q else 0.0
            mask128_f32 = const.tile([P, P], f32)
            nc.gpsimd.memset(mask128_f32[:], 1.0)
            nc.gpsimd.affine_select(
                out=mask128_f32[:],
                in_=mask128_f32[:],
                compare_op=mybir.AluOpType.is_ge,
                fill=0.0,
                base=0,
                pattern=[[1, P]],
                channel_multiplier=-1,
            )
            mask128 = const.tile([P, P], MM_DT)
            nc.vector.tensor_copy(mask128[:], mask128_f32[:])

            pools = (xt_pool, qkvt_pool, vaug_pool, pt_pool,
                     atn_pool, out_pool, small_pool, ps_mm, ps_s, ps_o)
            consts = (wqkv_sb, wp_sb, bqkv_sb, ident2, mask128,
                      x_d, out_d, xt_pre)
            for it in range(iters):
                _emit_body(nc, tc, pools, consts, it, phases)

    nc.compile()
    return nc


_CACHE = {}


def get_program(iters=1, phases='full'):
    key = (iters, phases)
    if key not in _CACHE:
        _CACHE[key] = _build_program(iters, phases)
    return _CACHE[key]


def make_in_maps(hidden_states, c_attn_w, c_attn_b, c_proj_w):
    import ml_dtypes

    bf = ml_dtypes.bfloat16
    x = np.asarray(hidden_states, dtype=np.float32).reshape(T, D)
    xt = np.ascontiguousarray(x.T.astype(bf))      # X^T [D, T] bf16
    wa = np.asarray(c_attn_w, dtype=np.float32)
    ba = np.asarray(c_attn_b, dtype=np.float32)
    wp = np.asarray(c_proj_w, dtype=np.float32)
    in_maps = []
    for c in range(N_CORES):
        lo, hi = c * P, (c + 1) * P
        w_qkv = np.ascontiguousarray(
            np.concatenate(
                [wa[:, lo:hi], wa[:, D + lo : D + hi], wa[:, 2 * D + lo : 2 * D + hi]],
                axis=1,
            ).astype(bf)
        )
        b_qkv = np.ascontiguousarray(
            np.concatenate([ba[lo:hi], ba[D + lo : D + hi], ba[2 * D + lo : 2 * D + hi]])
        )
        w_proj = np.ascontiguousarray(wp[lo:hi, :].astype(bf))
        in_maps.append({"x": xt, "w_qkv": w_qkv, "b_qkv": b_qkv, "w_proj": w_proj})
    return in_maps


def kernel(hidden_states, c_attn_w, c_attn_b, c_proj_w, c_proj_b):
    nc = get_program()
    in_maps = make_in_maps(hidden_states, c_attn_w, c_attn_b, c_proj_w)
    res = run_bass_kernel_spmd(nc, in_maps, list(range(N_CORES)))
    # unshard: row-parallel projection partials sum (fp32) + bias
    acc = np.zeros((T, D), dtype=np.float32)
    for c in range(N_CORES):
        acc += np.asarray(res.results[c]["out"], dtype=np.float32)
    acc += np.asarray(c_proj_b, dtype=np.float32)[None, :]
    return acc.reshape(B, S, D).astype(np.float32)


if __name__ == "__main__":
    rng = np.random.default_rng(0)
    hs = rng.standard_normal((B, S, D), dtype=np.float32)
    wa = rng.standard_normal((D, 3 * D), dtype=np.float32) * 0.02
    ba = rng.standard_normal((3 * D,), dtype=np.float32) * 0.02
    wp = rng.standard_normal((D, D), dtype=np.float32) * 0.02
    bp = rng.standard_normal((D,), dtype=np.float32) * 0.02
    out = kernel(hs, wa, ba, wp, bp)
    print("out", out.shape, out.dtype, float(np.abs(out).max()))
